# revision 1
# baseline (speedup 1.0000x reference)
"""Cost-volume kernel for Trainium2 (8 NeuronCores, batch-parallel).

Problem: cost[b, o=(dy,dx), h, w] = PReLU(mean_c(c1[b,c,h,w] *
         pad(warped)[b,c,h+dy,w+dx]), alpha), 81 offsets (9x9), zero pad 4.

Strategy per core (one batch element per NeuronCore):
  - Image tiled 16x8 pixels (th x tw), M=128 pixel tile, b-major partition
    order (m = b8*16 + a).
  - TensorE computes a "gram" tile against the 24x16 warped halo:
    PSUM[m, n] = sum_c c1[c, p_m] * wpad[c, halo_n]  (K=128+64 chunks,
    N=384, bf16 inputs, fp32 accumulate).
  - The 81 cost entries of pixel (a, b8) live at n = (a+dy)*16 + (b8+dx),
    a sheared per-partition window that no SBUF AP can express (partition
    steps cannot carry byte remainders), so the device writes the
    partition-uniform superset window [a*16, a*16+144) for each row-group
    a (partitions {a+16*b8}), and the host finishes with a cheap numpy
    diagonal gather + PReLU + 1/192 scale.
"""

import numpy as np

B, C, H, W = 8, 192, 128, 160
R = 4
TH, TW = 16, 8                    # pixel tile
HH, HWW = TH + 2 * R, TW + 2 * R  # halo 24 x 16
NCOL = HH * HWW                   # 384 matmul free dim
BANDS = H // TH                   # 8 row bands
TPB = W // TW                     # 20 tiles per band
WIN = 2 * R * HWW + TW + 2 * R    # 144 per-a superset window
PH, PW = H + 2 * R, W + 2 * R     # padded 136 x 168
K0, K1 = 96, 96                   # contraction chunks
GB = 2                            # bands per staged group
NGRP = BANDS // GB                # 4 staged groups
WROWS = PH // 2 + R               # 72 rows per wpad half (8-row overlap)

_CACHE = {}


def _build():
    if "nc" in _CACHE:
        return _CACHE["nc"]
    import sys
    if "/opt/trn_rl_repo" not in sys.path:
        sys.path.insert(0, "/opt/trn_rl_repo")
    import concourse.mybir as mybir
    import concourse.tile as tile
    from concourse import bacc
    from concourse.bass import AP

    nc = bacc.Bacc(None, target_bir_lowering=False)
    bf16 = mybir.dt.bfloat16
    f32 = mybir.dt.float32

    # c1 pre-tiled on host: [C, band, t, m], m = b8*16 + a
    c1_d = nc.dram_tensor("c1b", [C, H * W], bf16, kind="ExternalInput")
    wp_d = nc.dram_tensor("wpad", [C, PH * PW], bf16, kind="ExternalInput")
    go_d = nc.dram_tensor("gout", [NGRP * TH, TW * GB * TPB * WIN], bf16,
                          kind="ExternalOutput")

    with tile.TileContext(nc) as tc:
        with (
            tc.tile_pool(name="wp", bufs=1) as wp_pool,
            tc.tile_pool(name="c1", bufs=2) as c1_pool,
            tc.tile_pool(name="st", bufs=2) as st_pool,
            tc.tile_pool(name="ps", bufs=4, space="PSUM") as ps_pool,
        ):
            # persistent padded warped: 2 row-halves x 2 channel chunks,
            # all on the SP (sync) HWDGE ring so c1 loads don't queue
            # behind them.
            wp_sb = {}
            for half in range(2):
                row0 = half * (PH - WROWS)  # 0 or 64
                for k, (ks, kn) in enumerate(((0, K0), (K0, K1))):
                    t = wp_pool.tile([kn, WROWS * PW], bf16,
                                     tag=f"wp{half}{k}")
                    nc.sync.dma_start(
                        t[:], wp_d[ks:ks + kn,
                                   row0 * PW:(row0 + WROWS) * PW])
                    wp_sb[(half, k)] = t

            def load_c1(band):
                # c1 band loads on the ACT (scalar) HWDGE ring
                tiles = []
                for k, (ks, kn) in enumerate(((0, K0), (K0, K1))):
                    t = c1_pool.tile([kn, TPB * 128], bf16, tag=f"c1_{k}")
                    nc.scalar.dma_start(
                        t[:], c1_d[ks:ks + kn,
                                   band * TPB * 128:(band + 1) * TPB * 128])
                    tiles.append(t)
                return tiles

            # PE warm-up burst during the initial DMA window: ~5us of
            # back-to-back dummy matmuls flips the HAM clock gate to 2.4GHz
            # before the real stream starts.
            warm = c1_pool.tile([128, 512], bf16, tag="warm")
            nc.gpsimd.memset(warm[:], 0.0)
            for _ in range(12):
                ps_w = ps_pool.tile([128, 1024], f32, tag="ps")
                nc.tensor.matmul(ps_w[:, 0:512], warm[:, 0:128],
                                 warm[:, 0:512], start=True, stop=True)

            c1_cur = load_c1(0)
            for grp in range(NGRP):
                staged = st_pool.tile([128, GB * TPB * NCOL], bf16,
                                      tag="staged")
                sap0 = staged[:]
                srow = sap0.ap[0][0]

                for bb in range(GB):
                    band = grp * GB + bb
                    r0 = band * TH
                    half = 0 if band < BANDS // 2 else 1
                    prow0 = half * (PH - WROWS)
                    c1_sb = c1_cur
                    # prefetch next band ahead of this band's copies
                    if band + 1 < BANDS:
                        c1_cur = load_c1(band + 1)

                    for tp in range(TPB // 2):
                        ps = ps_pool.tile([128, 1024], f32, tag="ps")
                        for hf in range(2):
                            t_i = 2 * tp + hf
                            c0 = t_i * TW
                            for k, kn in enumerate((K0, K1)):
                                a1 = c1_sb[k][:]
                                lhsT = AP(a1.tensor,
                                          a1.offset + t_i * 128,
                                          [[a1.ap[0][0], kn], [1, 128]])
                                a2 = wp_sb[(half, k)][:]
                                rhs = AP(a2.tensor,
                                         a2.offset + (r0 - prow0) * PW + c0,
                                         [[a2.ap[0][0], kn],
                                          [PW, HH], [1, HWW]])
                                nc.tensor.matmul(
                                    ps[:, hf * 512:hf * 512 + NCOL],
                                    lhsT, rhs,
                                    start=(k == 0), stop=(k == 1))
                        # one copy moves both tiles' grams; DVE/ACT split
                        pap = ps[:]
                        src2 = AP(pap.tensor, pap.offset,
                                  [[pap.ap[0][0], 128], [512, 2],
                                   [1, NCOL]])
                        d0 = (bb * TPB + 2 * tp) * NCOL
                        dst2 = staged[:, d0:d0 + 2 * NCOL]
                        if tp % 5 < 3:
                            nc.vector.tensor_copy(dst2, src2)
                        else:
                            nc.scalar.copy(dst2, src2)

                # 16 per-a out-DMAs for the 2-band group, alternating rings
                gap = go_d[:]
                for a in range(TH):
                    src = AP(sap0.tensor,
                             sap0.offset + a * srow + a * HWW,
                             [[TH * srow, TW], [NCOL, GB * TPB], [1, WIN]])
                    dst = AP(gap.tensor,
                             gap.offset + (grp * TH + a)
                             * (TW * GB * TPB * WIN),
                             [[GB * TPB * WIN, TW], [WIN, GB * TPB],
                              [1, WIN]])
                    nc.sync.dma_start(dst, src)

    nc.finalize()
    _CACHE["nc"] = nc
    return nc


def kernel(c1, warped, alpha):
    import sys
    if "/opt/trn_rl_repo" not in sys.path:
        sys.path.insert(0, "/opt/trn_rl_repo")
    import ml_dtypes
    from concourse.bass_utils import run_bass_kernel_spmd

    nc = _build()
    bf = ml_dtypes.bfloat16

    in_maps = []
    for b in range(B):
        wpad = np.zeros((C, PH, PW), np.float32)
        wpad[:, R:R + H, R:R + W] = warped[b]
        # tile c1: [C, band, a, t, b8] -> [C, band, t, b8, a]; m = b8*16 + a
        c1t = np.asarray(c1[b]).reshape(C, BANDS, TH, TPB, TW)
        c1t = np.ascontiguousarray(c1t.transpose(0, 1, 3, 4, 2))
        in_maps.append({
            "c1b": c1t.reshape(C, H * W).astype(bf),
            "wpad": wpad.reshape(C, PH * PW).astype(bf),
        })

    import os
    trace = bool(int(os.environ.get("COSTVOL_TRACE", "0")))
    res = run_bass_kernel_spmd(nc, in_maps, core_ids=list(range(B)),
                               trace=trace)
    if trace:
        _CACHE["last_exec_time_ns"] = res.exec_time_ns

    # host-side: diagonal gather + mean + PReLU
    a_val = float(np.asarray(alpha).reshape(-1)[0])
    dy, dx = np.meshgrid(np.arange(9), np.arange(9), indexing="ij")
    oidx = (dy * HWW + dx).reshape(-1)                      # [81]
    jidx = np.arange(TW)[:, None] + oidx[None, :]           # [b8, 81]

    out = np.empty((B, 81, H, W), np.float32)
    for b in range(B):
        g = np.asarray(res.results[b]["gout"]).astype(np.float32)
        # [grp*16+a, b8, band2, t, j]
        g = g.reshape(NGRP, TH, TW, GB, TPB, WIN)
        got = np.take_along_axis(
            g, jidx[None, None, :, None, None, :], axis=5)
        # -> [81, grp, band2, a, t, b8] -> [81, h, w]
        cost = got.transpose(5, 0, 3, 1, 4, 2).reshape(81, H, W) * (1.0 / C)
        out[b] = np.where(cost >= 0, cost, a_val * cost)
    return out



# revision 4
# speedup vs baseline: 1.2027x; 1.2027x over previous
"""Cost-volume kernel for Trainium2 (8 NeuronCores, batch-parallel). v2

Problem: cost[b, o=(dy,dx), h, w] = PReLU(mean_c(c1[b,c,h,w] *
         pad(warped)[b,c,h+dy,w+dx]), alpha), 81 offsets (9x9), zero pad 4.

Strategy per core (one batch element per NeuronCore):
  - Image tiled 16x8 pixels, M=128 pixels per matmul tile.  Pixel->partition
    map is cluster-major: p = (a//4)*32 + b8*4 + (a%4)  (a=row-in-tile,
    b8=col-in-tile).  TensorE computes the gram of the pixel tile against its
    24x16 halo: PSUM[p, n] = sum_c c1[c, p] * wpad[c, halo_n], n = hh*16+ww,
    K = 96+96 chunks, bf16 in / fp32 acc.
  - The 81 cost entries of pixel (a, b8) live at n = (a+dy)*16 + (b8+dx).
    For a cluster c (a in [4c, 4c+4)) the union of all windows is the
    partition-uniform range [c*64, c*64+192) -- and clusters are 32
    CONSECUTIVE partitions, so a single engine copy per cluster compacts
    the gram into a per-partition-contiguous window buffer.  The out-DMA
    is then 1 descriptor per partition (7.7KB) instead of 288B shreds.
  - Host finishes with a cheap numpy diagonal gather (192 -> 81 per pixel,
    j = (a%4 + dy)*16 + b8 + dx), PReLU and the 1/192 mean scale.
"""

import numpy as np

B, C, H, W = 8, 192, 128, 160
R = 4
TH, TW = 16, 8                    # pixel tile
HH, HWW = TH + 2 * R, TW + 2 * R  # halo 24 x 16
NCOL = HH * HWW                   # 384 matmul free dim
BANDS = H // TH                   # 8 row bands
TPB = W // TW                     # 20 tiles per band
PH, PW = H + 2 * R, W + 2 * R     # padded 136 x 168
K0, K1 = 96, 96                   # contraction chunks
NCLUST = 4                        # a-clusters per tile (4 rows each)
WIN = (4 + 8) * HWW               # 192: 12 halo rows x 16 cols per cluster
NSEC = 8                          # wpad row sections
SECR = PH // NSEC                 # 17 rows per section

_CACHE = {}


def _build():
    if "nc" in _CACHE:
        return _CACHE["nc"]
    import sys
    if "/opt/trn_rl_repo" not in sys.path:
        sys.path.insert(0, "/opt/trn_rl_repo")
    import concourse.mybir as mybir
    import concourse.tile as tile
    from concourse import bacc
    from concourse.bass import AP

    nc = bacc.Bacc(None, target_bir_lowering=False)
    bf16 = mybir.dt.bfloat16
    f32 = mybir.dt.float32

    # c1 pre-tiled on host: [C, band, t, m], m = (a//4)*32 + b8*4 + a%4
    c1_d = nc.dram_tensor("c1b", [C, H * W], bf16, kind="ExternalInput")
    wp_d = nc.dram_tensor("wpad", [C, PH * PW], bf16, kind="ExternalInput")
    go_d = nc.dram_tensor("gout", [BANDS * 128, TPB * WIN], bf16,
                          kind="ExternalOutput")

    with tile.TileContext(nc) as tc:
        with (
            tc.tile_pool(name="wp", bufs=1) as wp_pool,
            tc.tile_pool(name="c1", bufs=3) as c1_pool,
            tc.tile_pool(name="st", bufs=2) as st_pool,
            tc.tile_pool(name="wn", bufs=2) as wn_pool,
            tc.tile_pool(name="ps", bufs=4, space="PSUM") as ps_pool,
        ):
            # persistent padded warped, full-resident, one tile per k-chunk,
            # loaded in NSEC row sections so band 0 can start early.
            wp_sb = []
            for k, (ks, kn) in enumerate(((0, K0), (K0, K1))):
                t = wp_pool.tile([kn, PH * PW], bf16, tag=f"wp{k}")
                wp_sb.append(t)
            for s in range(NSEC):
                for k, (ks, kn) in enumerate(((0, K0), (K0, K1))):
                    nc.sync.dma_start(
                        wp_sb[k][:, s * SECR * PW:(s + 1) * SECR * PW],
                        wp_d[ks:ks + kn,
                             s * SECR * PW:(s + 1) * SECR * PW])

            def load_c1(band):
                # c1 band loads on the ACT (scalar) HWDGE ring
                tiles = []
                for k, (ks, kn) in enumerate(((0, K0), (K0, K1))):
                    t = c1_pool.tile([kn, TPB * 128], bf16, tag=f"c1_{k}")
                    nc.scalar.dma_start(
                        t[:], c1_d[ks:ks + kn,
                                   band * TPB * 128:(band + 1) * TPB * 128])
                    tiles.append(t)
                return tiles

            # PE warm-up burst during the initial DMA window: ~5us of
            # back-to-back dummy matmuls flips the HAM clock gate to 2.4GHz
            # before the real stream starts.
            warm = c1_pool.tile([128, 512], bf16, tag="warm")
            nc.gpsimd.memset(warm[:], 0.0)
            for _ in range(12):
                ps_w = ps_pool.tile([128, 1024], f32, tag="ps")
                nc.tensor.matmul(ps_w[:, 0:512], warm[:, 0:128],
                                 warm[:, 0:512], start=True, stop=True)

            c1_bufs = [load_c1(0), load_c1(1)]
            for band in range(BANDS):
                r0 = band * TH
                c1_sb = c1_bufs.pop(0)
                if band + 2 < BANDS:
                    c1_bufs.append(load_c1(band + 2))

                staged = st_pool.tile([128, TPB * NCOL], bf16, tag="staged")
                sap0 = staged[:]
                srow = sap0.ap[0][0]

                for tp in range(TPB // 2):
                    ps = ps_pool.tile([128, 1024], f32, tag="ps")
                    for hf in range(2):
                        t_i = 2 * tp + hf
                        c0 = t_i * TW
                        for k, kn in enumerate((K0, K1)):
                            a1 = c1_sb[k][:]
                            lhsT = AP(a1.tensor,
                                      a1.offset + t_i * 128,
                                      [[a1.ap[0][0], kn], [1, 128]])
                            a2 = wp_sb[k][:]
                            rhs = AP(a2.tensor,
                                     a2.offset + r0 * PW + c0,
                                     [[a2.ap[0][0], kn],
                                      [PW, HH], [1, HWW]])
                            nc.tensor.matmul(
                                ps[:, hf * 512:hf * 512 + NCOL],
                                lhsT, rhs,
                                start=(k == 0), stop=(k == 1))
                    # one copy moves both tiles' grams; DVE/ACT split
                    pap = ps[:]
                    src2 = AP(pap.tensor, pap.offset,
                              [[pap.ap[0][0], 128], [512, 2],
                               [1, NCOL]])
                    d0 = 2 * tp * NCOL
                    dst2 = staged[:, d0:d0 + 2 * NCOL]
                    if tp % 2 == 0:
                        nc.vector.tensor_copy(dst2, src2)
                    else:
                        nc.scalar.copy(dst2, src2)

                # compact: per cluster of 32 consecutive partitions, the
                # window [c*64, c*64+192) of each tile's 384-block
                win = wn_pool.tile([128, TPB * WIN], bf16, tag="win")
                wap = win[:]
                wrow = wap.ap[0][0]
                for cc in range(NCLUST):
                    src = AP(sap0.tensor,
                             sap0.offset + cc * 32 * srow + cc * 64,
                             [[srow, 32], [NCOL, TPB], [1, WIN]])
                    dst = AP(wap.tensor,
                             wap.offset + cc * 32 * wrow,
                             [[wrow, 32], [WIN, TPB], [1, WIN]])
                    if cc % 2 == 0:
                        nc.vector.tensor_copy(dst, src)
                    else:
                        nc.scalar.copy(dst, src)

                # out: one descriptor per partition (TPB*WIN*2 = 7680B)
                gap = go_d[:]
                src = AP(wap.tensor, wap.offset,
                         [[wrow, 128], [1, TPB * WIN]])
                dst = AP(gap.tensor,
                         gap.offset + band * 128 * (TPB * WIN),
                         [[TPB * WIN, 128], [1, TPB * WIN]])
                nc.gpsimd.dma_start(dst, src)

    nc.finalize()
    _CACHE["nc"] = nc
    return nc


def kernel(c1, warped, alpha):
    import sys
    if "/opt/trn_rl_repo" not in sys.path:
        sys.path.insert(0, "/opt/trn_rl_repo")
    import ml_dtypes
    from concourse.bass_utils import run_bass_kernel_spmd

    nc = _build()
    bf = ml_dtypes.bfloat16

    in_maps = []
    for b in range(B):
        wpad = np.zeros((C, PH, PW), np.float32)
        wpad[:, R:R + H, R:R + W] = warped[b]
        # tile c1: [C, band, a, t, b8] -> [C, band, t, c, b8, q]
        # column m = c*32 + b8*4 + q, where a = 4c + q
        c1t = np.asarray(c1[b]).reshape(C, BANDS, NCLUST, 4, TPB, TW)
        c1t = np.ascontiguousarray(c1t.transpose(0, 1, 4, 2, 5, 3))
        in_maps.append({
            "c1b": c1t.reshape(C, H * W).astype(bf),
            "wpad": wpad.reshape(C, PH * PW).astype(bf),
        })

    import os
    trace = bool(int(os.environ.get("COSTVOL_TRACE", "0")))
    res = run_bass_kernel_spmd(nc, in_maps, core_ids=list(range(B)),
                               trace=trace)
    if trace:
        _CACHE["last_exec_time_ns"] = res.exec_time_ns

    # host-side: diagonal gather + mean + PReLU
    a_val = float(np.asarray(alpha).reshape(-1)[0])
    dy, dx = np.meshgrid(np.arange(9), np.arange(9), indexing="ij")
    dy = dy.reshape(-1)
    dx = dx.reshape(-1)                                      # [81]
    qq = np.arange(4)[:, None, None]
    bb8 = np.arange(TW)[None, :, None]
    # j = (q+dy)*16 + (b8+dx), shape [q, b8, 81]
    jidx = (qq + dy[None, None, :]) * HWW + bb8 + dx[None, None, :]

    out = np.empty((B, 81, H, W), np.float32)
    for b in range(B):
        g = np.asarray(res.results[b]["gout"]).astype(np.float32)
        # [band, c, b8, q, t, j]
        g = g.reshape(BANDS, NCLUST, TW, 4, TPB, WIN)
        got = np.take_along_axis(
            g, jidx.transpose(1, 0, 2)[None, None, :, :, None, :], axis=5)
        # got: [band, c, b8, q, t, 81] -> [81, band, c, q, t, b8]
        cost = got.transpose(5, 0, 1, 3, 4, 2).reshape(81, H, W) * (1.0 / C)
        out[b] = np.where(cost >= 0, cost, a_val * cost)
    return out
